# revision 21
# baseline (speedup 1.0000x reference)
"""Barycentric-coordinates KNN kernel for Trainium2 (8 NeuronCores).

Pipeline (per core = one (batch, half-of-V) pair; 8 cores cover 4 batches x 2
halves):
  Phase 1 (device): value matrix 2q.p - |p|^2 via fp32r TensorE matmuls
    (monotone in -d^2 per query row) into two PSUM tiles (cols 0..2047 and
    2048..4095); pairwise MAX fold of the two halves (column ambiguity is
    resolved on the host, which re-ranks both columns of each surviving
    slot); 6-bit in-chunk index packed into the low mantissa bits; DVE max8
    per 64-column folded chunk -> 256 candidate slots/row -> 512 host
    candidates. Fold/pack work is split between engines per v-tile
    (scalar-copy + gpsimd path vs DVE-fold path) to balance busy time.
  Host: decode candidates, exact fp32 d^2 re-rank to the true top-33
    (value asc, index asc), neighbor gather, SHOT weight normalization.
  Phase 2 (device): weighted 3x3 covariance eigensolver (Newton on the
    characteristic cubic + cross products), SHOT sign disambiguation,
    tangent-plane log map; per v-tile the template-cell value matrix
    VAL[row,(j,a,k)] = (dd_k^2+C) - 2 r_j (px_k cos_a + py_k sin_a) is
    built ON THE TENSOR ENGINE: PE-transpose of [px;py] -> lhsT [96,128]
    (ppc^T rows come pre-transposed from the host), one K=96 fp32r matmul
    against a constant sparse coefficient matrix G [96, 1280]; scalar
    copies PSUM->SBUF, gpsimd packs the 5-bit k-slot, DVE max8 per cell
    gives the 3 nearest (packed keys are negative so max8 ranks ascending
    with ties to the smaller k, matching the reference tie-break).
  Host: decode k-slots, gather projections, barycentric weights, assemble
    (4, 4096, 5, 8, 3, 2) output.
"""
import sys

sys.path.insert(0, "/opt/trn_rl_repo")

import numpy as np
from contextlib import ExitStack

import concourse.bass as bass
import concourse.mybir as mybir
import concourse.tile as tile
from concourse.bass_utils import run_bass_kernel_spmd
from concourse.tile import ScopedClock

f32 = np.float32
AF = mybir.ActivationFunctionType
ALU = mybir.AluOpType
DT = mybir.dt
AX = mybir.AxisListType

B, V, K = 4, 4096, 32
HALF = V // 2            # queries per core
NT = HALF // 128         # 16 v-tiles per core
CW = 64                  # phase-1 folded chunk width
NCH = 2048 // CW         # 32 chunks per folded row
CAND = NCH * 8           # 256 candidate slots per row (x2 columns on host)
R, A = 5, 8
NCELL = R * A            # 40 template cells
EPS = 1e-8
CKEY = 0.015625          # key offset: VAL = CKEY + dd^2 - 2 p.t > 0
TEMPLATE_RADIUS = 0.09
# ring radii exactly as create_template computes them in fp32
_RRJ = (f32(TEMPLATE_RADIUS)
        * (np.arange(1, R + 1, dtype=f32) / f32(R))).astype(f32)

# Quantized-add index packing (Pool engine has no bitwise/max ALU ops):
# adding BIG forces fp32 RNE onto a fixed absolute grid (the ulp of the
# biased domain), subtracting BIG back is exact (Sterbenz), and the slot
# index is then packed by ADDING idx*s where the whole index span stays
# below one grid step. Keys decode exactly: round(key / s) = q*(g/s) + idx.
QB1 = 192.0               # phase 1: VAL in [-3,2] -> [189,194), ulp g1 = 2^-16
S1 = 2.0 ** -22          # 64 slots * 2^-22 < 2^-16; ulp(|VAL|<=3) = 2^-22
QB2 = 12.0                # phase 2: VAL in [0,0.25) -> [12,12.25), g2 = 2^-20
S2 = 2.0 ** -25          # 32 slots * 2^-25 = 2^-20; ulp(0.25) = 2^-25

# ---------------------------------------------------------------------------
# Tile-framework workaround: walrus rejects instructions carrying more than a
# couple of sync waits. Spread extras across single-wait NOPs.
# ---------------------------------------------------------------------------


def _patched_drain_and_barrier(self, tick_clock, wait_clock):
    probe = self.nc.sync.nop(nofuse=True)
    wait_clock.add_sem_waits(probe.ins, ScopedClock({None: tick_clock.global_clock}))
    sync_info = probe.ins.sync_info
    waits = list(sync_info.on_wait or []) if sync_info is not None else []
    if len(waits) > 1:
        sync_info.on_wait = waits[:1]
        for i in range(1, len(waits)):
            extra = self.nc.sync.nop(nofuse=True)
            if extra.ins.sync_info is None:
                extra.ins.sync_info = mybir.SyncInfo(on_wait=[waits[i]], on_update=[])
            else:
                extra.ins.sync_info.on_wait = [waits[i]]
    self.nc.sync.drain()
    self.nc.all_engine_barrier()
    assert self.sems is not None
    popped = self.nc._tile_sem_poison_stack.pop()
    assert popped is self._sem_poison
    self.nc.clear_and_free_semaphores(list(self.sems.allocated().values()))
    self.nc.all_engine_barrier()


tile.TileContext._drain_and_barrier = _patched_drain_and_barrier


def split_sync_waits(nc, max_waits=1):
    for f in nc.m.functions:
        for b in f.blocks:
            new_list = []
            dirty = False
            for ins in b.instructions:
                si = ins.sync_info
                waits = list(si.on_wait) if (si is not None and si.on_wait) else []
                if len(waits) > max_waits:
                    dirty = True
                    extras, keep = waits[:-max_waits], waits[-max_waits:]
                    for j in range(0, len(extras), max_waits):
                        nop = mybir.InstNoOp(
                            name=f"I-wsplit-{nc.next_id()}", engine=ins.engine
                        )
                        nop.sync_info = mybir.SyncInfo(
                            on_wait=extras[j : j + max_waits], on_update=[]
                        )
                        new_list.append(nop)
                    si.on_wait = keep
                new_list.append(ins)
            if dirty:
                b.instructions = new_list


# ---------------------------------------------------------------------------
# Phase 1 program
# ---------------------------------------------------------------------------


def build_phase1():
    nc = bass.Bass()
    _register_consts(nc, [QB1, -QB1])
    pt4 = nc.declare_dram_parameter("pt4", [4, V], DT.float32r, isOutput=False)
    qt4 = nc.declare_dram_parameter("qt4", [4, HALF], DT.float32r, isOutput=False)
    cidx_i = nc.declare_dram_parameter("cidx", [128, 2048], DT.float32,
                                       isOutput=False)
    cand_o = nc.declare_dram_parameter("cand", [HALF, CAND], DT.float32, isOutput=True)

    with tile.TileContext(nc) as tc, ExitStack() as ctx:
        cpool = ctx.enter_context(tc.tile_pool(name="const", bufs=1))
        kpool = ctx.enter_context(tc.tile_pool(name="keys", bufs=3))
        opool = ctx.enter_context(tc.tile_pool(name="cand", bufs=4))
        ppool = ctx.enter_context(tc.tile_pool(name="psum", bufs=1, space="PSUM"))

        pt = cpool.tile([4, V], DT.float32r)
        qt = cpool.tile([4, HALF], DT.float32r)
        nc.sync.dma_start(qt[:], qt4[:])
        nc.sync.dma_start(pt[:, 2048:V], pt4[:, 2048:V])
        nc.sync.dma_start(pt[:, 0:2048], pt4[:, 0:2048])
        # idx*S1 per folded column, repeated per chunk
        CIDX = cpool.tile([128, 2048], DT.float32)
        nc.sync.dma_start(CIDX[:], cidx_i[:])
        B1T = cpool.tile([128, 1], DT.float32)
        nc.vector.memset(B1T[:], QB1)

        def emit_mms(t, psB0, psB1, psA0, psA1):
            # B halves first: their scalar staging frees the banks while the
            # A halves still stream, so the next tile's matmuls start sooner
            for ps, base in ((psB0, 2048), (psB1, 3072), (psA0, 0), (psA1, 1024)):
                for k4 in range(2):
                    nc.tensor.matmul(
                        ps[:, k4 * 512:(k4 + 1) * 512],
                        qt[:, t * 128:(t + 1) * 128],
                        pt[:, base + k4 * 512: base + (k4 + 1) * 512],
                        start=True, stop=True,
                    )

        def emit_front(t, psB0, psB1, psA0, psA1):
            # stage psB (+B1) via scalar, fold with (psA+B1) on DVE, recover
            # the grid-quantized value (-B1, exact) on scalar, add idx*S1 on
            # gpsimd
            SB = kpool.tile([128, 2048], DT.float32, tag="sb", bufs=2)
            nc.scalar.activation(SB[:, 0:1024], psB0[:], AF.Identity,
                                 bias=QB1, scale=1.0)
            nc.scalar.activation(SB[:, 1024:2048], psB1[:], AF.Identity,
                                 bias=QB1, scale=1.0)
            F = kpool.tile([128, 2048], DT.float32, tag="fold", bufs=2)
            nc.vector.scalar_tensor_tensor(
                out=F[:, 0:1024], in0=psA0[:], scalar=B1T[:],
                in1=SB[:, 0:1024], op0=ALU.add, op1=ALU.max)
            nc.vector.scalar_tensor_tensor(
                out=F[:, 1024:2048], in0=psA1[:], scalar=B1T[:],
                in1=SB[:, 1024:2048], op0=ALU.add, op1=ALU.max)
            F2 = kpool.tile([128, 2048], DT.float32, tag="f2", bufs=2)
            nc.scalar.activation(F2[:], F[:], AF.Identity, bias=-QB1, scale=1.0)
            KEY = kpool.tile([128, 2048], DT.float32, tag="key", bufs=4)
            nc.gpsimd.tensor_tensor(out=KEY[:], in0=F2[:], in1=CIDX[:],
                                    op=ALU.add)
            return KEY

        def emit_back(t, KEY):
            cv = opool.tile([128, CAND], DT.float32, tag="cv")
            for c in range(NCH):
                nc.vector.max(out=cv[:, c * 8:(c + 1) * 8],
                              in_=KEY[:, c * CW:(c + 1) * CW])
            nc.sync.dma_start(cand_o[t * 128:(t + 1) * 128, :], cv[:])

        keys = {}
        for t in range(NT):
            pss = [ppool.tile([128, 1024], DT.float32, space="PSUM",
                              tag=f"ps{i}", name=f"ps{i}")
                   for i in range(4)]
            emit_mms(t, *pss)
            keys[t] = emit_front(t, *pss)
            if t >= 1:
                emit_back(t - 1, keys.pop(t - 1))
        emit_back(NT - 1, keys.pop(NT - 1))

    split_sync_waits(nc)
    return nc


# ---------------------------------------------------------------------------
# Phase 2 program
# ---------------------------------------------------------------------------


def _register_consts(nc, values):
    for value in values:
        t = nc.alloc_sbuf_tensor(f"const-float32-{value}", [128, 1], DT.float32)
        nc.gpsimd.memset(t.ap(), value)
        nc.const_aps.aps[(DT.float32, value)] = t.ap()
    nc.all_engine_barrier()


def build_phase2():
    nc = bass.Bass()
    _register_consts(nc, [0.5, -3.0, 64.0, QB2, 1e-30, 1e-36])
    ngh_i = nc.declare_dram_parameter("ngh", [HALF, 96], DT.float32, isOutput=False)
    cov_i = nc.declare_dram_parameter("cov6", [HALF, 8], DT.float32, isOutput=False)
    dd_i = nc.declare_dram_parameter("dd", [HALF, K], DT.float32, isOutput=False)
    ppct_i = nc.declare_dram_parameter("ppct", [32, HALF], DT.float32,
                                       isOutput=False)
    g_i = nc.declare_dram_parameter("gmat", [96, NCELL * K], DT.float32,
                                    isOutput=False)
    idn_i = nc.declare_dram_parameter("idn", [128, 128], DT.float32,
                                      isOutput=False)
    m3_o = nc.declare_dram_parameter("m3o", [HALF, NCELL, 3], DT.float32,
                                     isOutput=True)
    pxy_o = nc.declare_dram_parameter("pxy", [HALF, 2, K], DT.float32,
                                      isOutput=True)

    with tile.TileContext(nc) as tc, ExitStack() as ctx:
        cp = ctx.enter_context(tc.tile_pool(name="const", bufs=1))
        sp = ctx.enter_context(tc.tile_pool(name="scratch", bufs=2))
        bp = ctx.enter_context(tc.tile_pool(name="bc", bufs=2))
        tpp = ctx.enter_context(tc.tile_pool(name="tpsum", bufs=2, space="PSUM"))
        vpp = ctx.enter_context(tc.tile_pool(name="vpsum", bufs=2, space="PSUM"))

        NGH = cp.tile([128, NT, 96], DT.float32)
        COV6 = cp.tile([128, NT, 8], DT.float32)
        DD = cp.tile([128, NT, K], DT.float32)
        IDN = cp.tile([128, 128], DT.float32)
        G = cp.tile([96, NCELL * K], DT.float32)
        # lhsT bank: [px;py] columns written per-tile from the PE transpose,
        # ppc^T rows DMA'd once from the host
        LTC = cp.tile([96, HALF], DT.float32)
        nc.sync.dma_start(COV6[:], cov_i[:].rearrange("(t p) c -> p t c", p=128))
        nc.sync.dma_start(NGH[:], ngh_i[:].rearrange("(t p) c -> p t c", p=128))
        nc.sync.dma_start(DD[:], dd_i[:].rearrange("(t p) c -> p t c", p=128))
        nc.sync.dma_start(IDN[:], idn_i[:])
        nc.sync.dma_start(G[:], g_i[:])
        nc.sync.dma_start(LTC[64:96, :], ppct_i[:])

        # low-5-bit slot id plus the sign bit: packed keys become negative
        # floats, so max8 ranks by ascending VAL with ties to the smaller k,
        # matching the reference tie-break
        KIOTA = cp.tile([128, NCELL, K], DT.int32)
        nc.gpsimd.iota(KIOTA[:], pattern=[[0, NCELL], [1, K]],
                       base=-2147483648, channel_multiplier=0)
        M32 = cp.tile([128, 1], DT.int32)
        nc.vector.memset(M32[:], -32)

        _tagn = [0]

        def nt_tile(pool=cp):
            _tagn[0] += 1
            return pool.tile([128, NT], DT.float32, tag=f"nt{_tagn[0]}",
                             name=f"nt{_tagn[0]}")

        def n2_tile(pool=cp):
            _tagn[0] += 1
            return pool.tile([128, 2 * NT], DT.float32, tag=f"n2{_tagn[0]}",
                             name=f"n2{_tagn[0]}")

        CXX = COV6[:, :, 0]
        CXY = COV6[:, :, 1]
        CXZ = COV6[:, :, 2]
        CYY = COV6[:, :, 3]
        CYZ = COV6[:, :, 4]
        CZZ = COV6[:, :, 5]

        # ---- eigensolver; scalar chain on (128, NT), then the two Newton
        # runs and the two eigenvector extractions merged into (128, 2*NT) ----
        def _ap(x):
            return x if isinstance(x, bass.AP) else x[:]

        def tt(dst, a, bb, op):
            nc.vector.tensor_tensor(out=_ap(dst), in0=_ap(a), in1=_ap(bb), op=op)

        def sq_act(dst, a):
            nc.scalar.activation(_ap(dst), _ap(a), AF.Square)

        # component weights [1,2,2,1,2,1] and diagonal Q mask, built on the
        # scalar/vector engines off the critical path
        WB = cp.tile([128, 6], DT.float32)
        nc.gpsimd.memset(WB[:], 1.0)
        nc.gpsimd.memset(WB[:, 1:3], 2.0)
        nc.gpsimd.memset(WB[:, 4:5], 2.0)
        QD6 = cp.tile([128, NT, 6], DT.float32)
        nc.gpsimd.memset(QD6[:], 0.0)
        Q = nt_tile()
        tt(Q, CXX, CYY, ALU.add)
        tt(Q, Q, CZZ, ALU.add)
        nc.vector.tensor_scalar_mul(Q[:], Q[:], 1.0 / 3.0)
        for dcol in (0, 3, 5):
            nc.scalar.copy(QD6[:, :, dcol], Q[:])
        # BALL = A - Q.I over all 6 components at once
        BALL = cp.tile([128, NT, 6], DT.float32)
        nc.vector.tensor_tensor(out=BALL[:], in0=COV6[:, :, 0:6], in1=QD6[:],
                                op=ALU.subtract)
        # P2 = sum_c w_c BALL_c^2
        SQ6 = cp.tile([128, NT, 6], DT.float32)
        nc.vector.tensor_tensor(out=SQ6[:], in0=BALL[:], in1=BALL[:],
                                op=ALU.mult)
        wb = WB[:].rearrange("p c -> p () c").to_broadcast([128, NT, 6])
        nc.vector.tensor_tensor(out=SQ6[:], in0=SQ6[:], in1=wb, op=ALU.mult)
        P2 = nt_tile()
        nc.vector.tensor_reduce(out=P2[:], in_=SQ6[:], axis=AX.X, op=ALU.add)
        T1 = nt_tile(sp)
        T2 = nt_tile(sp)
        PP = nt_tile()
        PPX = nt_tile()
        nc.vector.tensor_scalar_mul(PPX[:], P2[:], 1.0 / 6.0)

        def polished_sqrt(dst, x, tmp):
            # ACT Sqrt is ~7e-6; one Newton step s' = (s + x/s)/2 fixes it
            nc.scalar.activation(_ap(dst), _ap(x), AF.Sqrt)
            nc.vector.tensor_scalar_max(_ap(tmp), _ap(dst), 1e-30)
            nc.vector.reciprocal(_ap(tmp), _ap(tmp))
            nc.vector.tensor_tensor(out=_ap(tmp), in0=_ap(x), in1=_ap(tmp),
                                    op=ALU.mult)
            nc.vector.tensor_tensor(out=_ap(dst), in0=_ap(dst), in1=_ap(tmp),
                                    op=ALU.add)
            nc.vector.tensor_scalar_mul(_ap(dst), _ap(dst), 0.5)

        polished_sqrt(PP, PPX, T2)
        PINV = nt_tile()
        nc.vector.tensor_scalar_max(PINV[:], PP[:], 1e-20)
        nc.vector.reciprocal(PINV[:], PINV[:])
        # normalized B-hat, all 6 components in one op
        NBALL = cp.tile([128, NT, 6], DT.float32)
        pinvb = PINV[:].rearrange("p t -> p t ()").to_broadcast([128, NT, 6])
        nc.vector.tensor_tensor(out=NBALL[:], in0=BALL[:], in1=pinvb,
                                op=ALU.mult)
        NBXX = NBALL[:, :, 0]
        NBXY = NBALL[:, :, 1]
        NBXZ = NBALL[:, :, 2]
        NBYY = NBALL[:, :, 3]
        NBYZ = NBALL[:, :, 4]
        NBZZ = NBALL[:, :, 5]
        # det(B-hat)
        DET = nt_tile()
        sq_act(T1, NBYZ)                     # byz^2
        tt(T2, NBYY, NBZZ, ALU.mult)
        tt(T2, T2, T1, ALU.subtract)
        tt(DET, NBXX, T2, ALU.mult)          # + bxx (byy bzz - byz^2)
        tt(T1, NBXY, NBZZ, ALU.mult)
        tt(T2, NBYZ, NBXZ, ALU.mult)
        tt(T1, T1, T2, ALU.subtract)
        tt(T1, NBXY, T1, ALU.mult)
        tt(DET, DET, T1, ALU.subtract)       # - bxy (bxy bzz - byz bxz)
        tt(T1, NBXY, NBYZ, ALU.mult)
        tt(T2, NBYY, NBXZ, ALU.mult)
        tt(T1, T1, T2, ALU.subtract)
        tt(T1, NBXZ, T1, ALU.mult)
        tt(DET, DET, T1, ALU.add)            # + bxz (bxy byz - byy bxz)
        R2 = nt_tile()                       # 2r = det  clamped to [-2, 2]
        nc.vector.tensor_scalar_min(R2[:], DET[:], 2.0)
        nc.vector.tensor_scalar_max(R2[:], R2[:], -2.0)

        # merged Newton: halves [beta(+2.2) | beta(-2.2)] over (128, 2*NT)
        def dup(src):
            d = n2_tile()
            nc.scalar.copy(d[:, 0:NT], _ap(src))
            nc.scalar.copy(d[:, NT:2 * NT], _ap(src))
            return d

        # dups emitted early: the scalar engine fills them while the
        # vector engine runs the Newton iterations
        CXXD, CXYD, CXZD = dup(CXX), dup(CXY), dup(CXZ)
        CYYD, CYZD, CZZD = dup(CYY), dup(CYZ), dup(CZZ)
        PPD = dup(PP)
        QD = dup(Q)
        R2D = dup(R2)
        # [u | -u] for the odd-symmetry polynomial init: βmin(u) = -βmax(-u)
        R2S = n2_tile()
        nc.scalar.copy(R2S[:, 0:NT], R2[:])
        nc.scalar.activation(R2S[:, NT:2 * NT], R2[:], AF.Identity, scale=-1.0)
        T1D = n2_tile(sp)
        # cubic LS fit of the largest root of β³-3β-u on u ∈ [-2,2]
        # (max err 0.15), then 4 Newton steps -> <1e-6 away from double roots
        BETA = n2_tile()
        nc.vector.tensor_scalar(out=BETA[:], in0=R2S[:], scalar1=0.01574144,
                                scalar2=-0.03955863, op0=ALU.mult, op1=ALU.add)
        tt(BETA, BETA, R2S, ALU.mult)
        nc.vector.tensor_scalar_add(BETA[:], BETA[:], 0.15508261)
        tt(BETA, BETA, R2S, ALU.mult)
        nc.vector.tensor_scalar_add(BETA[:], BETA[:], 1.74024065)
        nc.scalar.activation(BETA[:, NT:2 * NT], BETA[:, NT:2 * NT],
                             AF.Identity, scale=-1.0)
        FV = n2_tile(sp)
        B2 = n2_tile(sp)
        for _ in range(4):
            tt(B2, BETA, BETA, ALU.mult)                  # β²
            tt(FV, B2, BETA, ALU.mult)                    # β³
            nc.vector.scalar_tensor_tensor(
                out=T1D[:], in0=BETA[:], scalar=3.0, in1=FV[:],
                op0=ALU.mult, op1=ALU.subtract)           # 3β - β³
            tt(T1D, T1D, R2D, ALU.add)                    # -f = 3β - β³ + 2r
            nc.vector.tensor_scalar(out=B2[:], in0=B2[:], scalar1=3.0,
                                    scalar2=-3.0, op0=ALU.mult, op1=ALU.add)
            nc.vector.tensor_scalar_max(B2[:], B2[:], 1e-8)
            nc.vector.reciprocal(B2[:], B2[:])
            tt(T1D, T1D, B2, ALU.mult)                    # -f/f'
            tt(BETA, BETA, T1D, ALU.add)                  # β - f/f'
        # LL = [λmax | λmin]
        LL = n2_tile()
        tt(LL, PPD, BETA, ALU.mult)
        tt(LL, LL, QD, ALU.add)

        # merged eigenvector extraction: halves [x-axis(λmax) | z-axis(λmin)]

        def gtt(dst, a, bb, op):
            nc.gpsimd.tensor_tensor(out=_ap(dst), in0=_ap(a), in1=_ap(bb), op=op)

        def evec2(lam):
            # columns of A - lam I; arithmetic on gpsimd (idle during the
            # eigen prologue), comparisons/reciprocals on vector, squares on
            # scalar
            D0, D1, D2 = n2_tile(sp), n2_tile(sp), n2_tile(sp)
            tt(D0, CXXD, lam, ALU.subtract)
            gtt(D1, CYYD, lam, ALU.subtract)
            tt(D2, CZZD, lam, ALU.subtract)
            m0 = (D0, CXYD, CXZD)
            m1 = (CXYD, D1, CYZD)
            m2 = (CXZD, CYZD, D2)

            def cross(u, v, op):
                rx, ry, rz = n2_tile(sp), n2_tile(sp), n2_tile(sp)
                tmp = n2_tile(sp)
                op(rx, u[1], v[2], ALU.mult)
                op(tmp, u[2], v[1], ALU.mult)
                op(rx, rx, tmp, ALU.subtract)
                op(ry, u[2], v[0], ALU.mult)
                op(tmp, u[0], v[2], ALU.mult)
                op(ry, ry, tmp, ALU.subtract)
                op(rz, u[0], v[1], ALU.mult)
                op(tmp, u[1], v[0], ALU.mult)
                op(rz, rz, tmp, ALU.subtract)
                return rx, ry, rz

            def norm2(c, op):
                n = n2_tile(sp)
                tmp = n2_tile(sp)
                op(n, c[0], c[0], ALU.mult)
                op(tmp, c[1], c[1], ALU.mult)
                op(n, n, tmp, ALU.add)
                op(tmp, c[2], c[2], ALU.mult)
                op(n, n, tmp, ALU.add)
                return n

            # run the prologue wide: DVE and gpsimd queues fill in parallel
            c01 = cross(m0, m1, tt)
            c12 = cross(m1, m2, gtt)
            c02 = cross(m0, m2, tt)
            n01, n02, n12 = norm2(c01, tt), norm2(c02, tt), norm2(c12, gtt)
            G1, G2, G3 = n2_tile(sp), n2_tile(sp), n2_tile(sp)
            tt(G1, n01, n02, ALU.is_ge)
            tt(G2, n01, n12, ALU.is_ge)
            tt(G1, G1, G2, ALU.mult)                    # pick01
            tt(G3, n02, n12, ALU.is_ge)
            U = n2_tile(sp)
            nc.vector.tensor_scalar(out=U[:], in0=G1[:], scalar1=-1.0, scalar2=1.0,
                                    op0=ALU.mult, op1=ALU.add)   # 1 - pick01
            tt(G2, U, G3, ALU.mult)                     # pick02
            nc.vector.tensor_scalar(out=G3[:], in0=G3[:], scalar1=-1.0, scalar2=1.0,
                                    op0=ALU.mult, op1=ALU.add)   # 1 - g3
            tt(G3, U, G3, ALU.mult)                     # pick12
            out = []
            for ci in range(3):
                op = (tt, gtt, tt)[ci]
                VC = n2_tile()
                tmp = n2_tile(sp)
                op(VC, c01[ci], G1, ALU.mult)
                op(tmp, c02[ci], G2, ALU.mult)
                op(VC, VC, tmp, ALU.add)
                op(tmp, c12[ci], G3, ALU.mult)
                op(VC, VC, tmp, ALU.add)
                out.append(VC)
            # only the z-half (λmin) needs unit scale: the log map is
            # homogeneous in the shared x/y scale (y = z × x), so the x-half
            # can stay unnormalized. Normalize z via scalar Rsqrt (~1e-5).
            n2v = norm2(out, tt)
            rn = n2_tile(sp)
            nc.scalar.activation(rn[:, NT:2 * NT], n2v[:, NT:2 * NT],
                                 AF.Sqrt, bias=1e-30, scale=1.0)
            nc.vector.tensor_scalar_max(rn[:, NT:2 * NT], rn[:, NT:2 * NT],
                                        1e-30)
            nc.vector.reciprocal(rn[:, NT:2 * NT], rn[:, NT:2 * NT])
            for VC in out:
                tt(VC[:, NT:2 * NT], VC[:, NT:2 * NT], rn[:, NT:2 * NT],
                   ALU.mult)
            return out

        AXD = evec2(LL)

        HNT = NT // 2

        def axbh(ap_, sl):
            return ap_[:, sl].rearrange("p t -> p t ()") \
                .to_broadcast([128, HNT, K])

        def bch(ap_):
            # ap_ is already half-width (128, HNT)
            return ap_.rearrange("p t -> p t ()").to_broadcast([128, HNT, K])

        def dot_axis_h(axes, sl, add_eng, prod_eng=None):
            # batched NGH . axis over half the tiles; products on gpsimd,
            # adds on the engine with slack for this half
            prod_eng = prod_eng or nc.gpsimd
            DST = cp.tile([128, HNT, K], DT.float32, tag=f"dot{_tagn[0]}",
                          name=f"dot{_tagn[0]}")
            _tagn[0] += 1
            TA = sp.tile([128, HNT, K], DT.float32, tag="dta")
            TB = sp.tile([128, HNT, K], DT.float32, tag="dtb")
            prod_eng.tensor_tensor(out=DST[:], in0=NGH[:, sl, 0:K],
                                   in1=bch(axes[0]), op=ALU.mult)
            prod_eng.tensor_tensor(out=TA[:], in0=NGH[:, sl, K:2 * K],
                                   in1=bch(axes[1]), op=ALU.mult)
            prod_eng.tensor_tensor(out=TB[:], in0=NGH[:, sl, 2 * K:3 * K],
                                   in1=bch(axes[2]), op=ALU.mult)
            add_eng.tensor_tensor(out=DST[:], in0=DST[:], in1=TA[:], op=ALU.add)
            add_eng.tensor_tensor(out=DST[:], in0=DST[:], in1=TB[:], op=ALU.add)
            return DST

        SG = cp.tile([128, NT, K], DT.float32)
        FX = nt_tile()
        FZ = nt_tile()
        YAX = [nt_tile() for _ in range(3)]
        PXY = cp.tile([128, NT, 2, K], DT.float32)
        SC = cp.tile([128, NT, K], DT.float32)
        U2 = cp.tile([128, NT, K], DT.float32)

        def emit_tail(h):
            # dots/signs/log-map for one half of the v-tiles, so the BC
            # chains of the first half can start while the second half runs
            sl = slice(h * HNT, (h + 1) * HNT)
            eng2 = nc.gpsimd if h else nc.vector
            xs = [AXD[c][:, h * HNT:(h + 1) * HNT] for c in range(3)]
            zs = [AXD[c][:, NT + h * HNT:NT + (h + 1) * HNT] for c in range(3)]
            if h == 0:
                # parallel queues: DOTX fully on DVE, DOTZ fully on gpsimd
                DOTX = dot_axis_h(xs, sl, nc.vector, prod_eng=nc.vector)
                DOTZ = dot_axis_h(zs, sl, nc.gpsimd, prod_eng=nc.gpsimd)
            else:
                DOTX = dot_axis_h(xs, sl, eng2)
                DOTZ = dot_axis_h(zs, sl, eng2)
            for DOT, F in ((DOTX, FX), (DOTZ, FZ)):
                nc.scalar.activation(SG[:, sl, :], DOT[:], AF.Sign)
                nc.vector.tensor_reduce(out=F[:, sl], in_=SG[:, sl, :],
                                        axis=AX.X, op=ALU.add)
                nc.scalar.activation(F[:, sl], F[:, sl], AF.Sign, bias=0.5,
                                     scale=1.0)
            for c in range(3):
                eng2.tensor_tensor(out=_ap(xs[c]), in0=_ap(xs[c]),
                                   in1=FX[:, sl], op=ALU.mult)
                eng2.tensor_tensor(out=_ap(zs[c]), in0=_ap(zs[c]),
                                   in1=FZ[:, sl], op=ALU.mult)
            eng2.tensor_tensor(out=DOTX[:], in0=DOTX[:], in1=axbh(FX, sl),
                               op=ALU.mult)
            # y = cross(z, x)
            ys = []
            for (i1, i2) in ((1, 2), (2, 0), (0, 1)):
                YC = YAX[len(ys)][:, sl]
                YT = nt_tile(sp)
                gtt(YC, zs[i1], xs[i2], ALU.mult)
                gtt(YT[:, sl], zs[i2], xs[i1], ALU.mult)
                gtt(YC, YC, YT[:, sl], ALU.subtract)
                ys.append(YC)
            DOTY = dot_axis_h(ys, sl, eng2)
            # log map into PXY: px = dotx * dd * rsqrt(dotx^2 + doty^2)
            # (homogeneous in the shared x/y axis scale)
            PXs = PXY[:, sl, 0, :]
            PYs = PXY[:, sl, 1, :]
            u2 = U2[:, sl, :]
            sc = SC[:, sl, :]
            nc.scalar.activation(u2, DOTX[:], AF.Square)
            nc.scalar.activation(sc, DOTY[:], AF.Square)
            eng2.tensor_tensor(out=u2, in0=u2, in1=sc, op=ALU.add)
            nc.scalar.activation(sc, u2, AF.Sqrt, bias=1e-36, scale=1.0)
            nc.vector.reciprocal(sc, sc)
            nc.gpsimd.tensor_tensor(out=sc, in0=sc, in1=DD[:, sl, :],
                                    op=ALU.mult)
            nc.gpsimd.tensor_tensor(out=PXs, in0=DOTX[:], in1=sc, op=ALU.mult)
            nc.gpsimd.tensor_tensor(out=PYs, in0=DOTY[:], in1=sc, op=ALU.mult)
            nc.sync.dma_start(
                pxy_o[h * HNT * 128:(h + 1) * HNT * 128]
                .rearrange("(t p) x k -> p t x k", p=128), PXY[:, sl])

        # ---- BC selection: PE builds VAL[row,(j,a,k)] via transpose +
        # full-fp32 matmul (the top-3 ranking is margin-sensitive at ~1e-7,
        # ruling out fp32r); scalar copies PSUM->SBUF; DVE packs the k-slot
        # into the low 5 mantissa bits and max8 per cell picks the 3
        # nearest. ----
        NC3 = NCELL * K  # 1280

        def emit_front2(t):
            TPS = tpp.tile([64, 128], DT.float32, space="PSUM", tag="tps")
            nc.tensor.transpose(TPS[:], PXY[:, t, :, :], IDN[:])
            nc.scalar.copy(LTC[0:64, t * 128:(t + 1) * 128], TPS[:])
            VPS = vpp.tile([128, 1536], DT.float32, space="PSUM", tag="vps")
            lhsT = LTC[:, t * 128:(t + 1) * 128]
            for n0 in range(0, NC3, 512):
                n1 = min(n0 + 512, NC3)
                nc.tensor.matmul(VPS[:, n0:n1], lhsT, G[:, n0:n1],
                                 start=True, stop=True)
            KEY = bp.tile([128, NCELL, K], DT.int32, tag="bkey", bufs=3)
            nc.vector.scalar_tensor_tensor(
                out=KEY[:], in0=VPS[:, 0:NC3].rearrange("p (c k) -> p c k", k=K)
                .bitcast(DT.int32), scalar=M32[:], in1=KIOTA[:],
                op0=ALU.bitwise_and, op1=ALU.bitwise_or)
            return KEY

        def emit_back2(t, KEY):
            M8 = bp.tile([128, NCELL, 8], DT.float32, tag="m8", bufs=4)
            keyv = KEY[:].bitcast(DT.float32)
            for c in range(NCELL):
                nc.vector.max(out=M8[:, c, :], in_=keyv[:, c, :])
            M3C = bp.tile([128, NCELL, 3], DT.float32, tag="m3c", bufs=4)
            nc.scalar.copy(M3C[:], M8[:, :, 0:3])
            nc.sync.dma_start(
                m3_o[t * 128:(t + 1) * 128, :, :]
                .rearrange("(t p) c s -> p t c s", p=128), M3C[:])

        emit_tail(0)
        keys = {}
        for t in range(HNT):
            keys[t] = emit_front2(t)
            if t >= 1:
                emit_back2(t - 1, keys.pop(t - 1))
        emit_tail(1)
        for t in range(HNT, NT):
            keys[t] = emit_front2(t)
            emit_back2(t - 1, keys.pop(t - 1))
        emit_back2(NT - 1, keys.pop(NT - 1))

    split_sync_waits(nc)
    return nc


# ---------------------------------------------------------------------------
# Host glue
# ---------------------------------------------------------------------------


def host_prep_phase1(vertices):
    """vertices (4, 4096, 3) -> list of 8 input maps."""
    cidx = np.broadcast_to(
        ((np.arange(2048) % CW) * S1).astype(f32)[None, :], (128, 2048))
    cidx = np.ascontiguousarray(cidx)
    maps = []
    for core in range(8):
        b, h = core // 2, core % 2
        verts = np.ascontiguousarray(vertices[b], dtype=f32)
        sq = (verts * verts).sum(-1, dtype=f32).astype(f32)
        pt4 = np.concatenate([verts.T, sq[None, :]], axis=0).astype(f32)
        Q = verts[h * HALF:(h + 1) * HALF]
        qt4 = np.concatenate([2.0 * Q.T, -np.ones((1, HALF), f32)],
                             axis=0).astype(f32)
        maps.append({"pt4": np.ascontiguousarray(pt4),
                     "qt4": np.ascontiguousarray(qt4), "cidx": cidx})
    return maps


_CHUNK_BASE = (np.arange(CAND, dtype=np.int64) // 8 * CW)


def host_merge(cand_keys, verts, h):
    """Decode folded-slot candidates (each slot -> 2 columns), exact fp32
    top-33 re-rank.

    -> nbr (HALF,32) int64, d (HALF,32), radius (HALF,)."""
    m = np.rint(cand_keys.astype(np.float64) * (1.0 / S1)).astype(np.int64)
    base = (m % CW) + _CHUNK_BASE[None, :]
    cand = np.concatenate([base, base + 2048], axis=1)        # (HALF, 512)
    sq = (verts * verts).sum(-1, dtype=f32).astype(f32)
    Q = verts[h * HALF:(h + 1) * HALF]
    qsq = sq[h * HALF:(h + 1) * HALF]
    pc = verts[cand]                                          # (HALF,512,3)
    dots = np.einsum("qck,qk->qc", pc, Q, dtype=f32).astype(f32)
    d2 = (sq[cand] + qsq[:, None] - 2.0 * dots).astype(f32)
    order = np.lexsort((cand, d2), axis=1)[:, :33]
    top = np.take_along_axis(cand, order, axis=1)
    d2t = np.take_along_axis(d2, order, axis=1)
    d33 = np.sqrt(np.maximum(d2t, 0.0)).astype(f32)
    return top[:, :32], d33[:, :32], d33[:, 32]


def host_prep_phase2(vertices, template, p1_results):
    """Build phase-2 input maps + per-core nbr tables from phase-1 outputs."""
    template = np.asarray(template, f32)
    tx = template[..., 0]                    # (R, A)
    ty = template[..., 1]
    rr = tx[:, 0].astype(f32)                # angle 0: sin=0, cos=1 -> r_j
    assert np.array_equal(rr, _RRJ), "template radii differ from compiled-in"
    cosv = (tx[0] / rr[0]).astype(f32)
    sinv = (ty[0] / rr[0]).astype(f32)
    # sparse coefficient matrix: VAL[(j,a,k)] = ppc_k - 2 r_j (cos_a px_k +
    # sin_a py_k); contraction rows 0..31 = px, 32..63 = py, 64..95 = ppc
    G = np.zeros((96, NCELL * K), f32)
    for j in range(R):
        for a in range(A):
            c0 = (j * A + a) * K
            cf = f32(-2.0) * rr[j] * cosv[a]
            sf = f32(-2.0) * rr[j] * sinv[a]
            for k in range(K):
                G[k, c0 + k] = cf
                G[32 + k, c0 + k] = sf
                G[64 + k, c0 + k] = 1.0
    G = np.ascontiguousarray(G)
    idn = np.ascontiguousarray(np.eye(128, dtype=f32))
    maps, nbrs = [], []
    for core in range(8):
        b, h = core // 2, core % 2
        verts = np.ascontiguousarray(vertices[b], dtype=f32)
        nbr, d, radius = host_merge(p1_results[core]["cand"], verts, h)
        Q = verts[h * HALF:(h + 1) * HALF]
        neigh = (verts[nbr] - Q[:, None, :]).astype(f32)          # (HALF, 32, 3)
        ngh = np.ascontiguousarray(neigh.transpose(0, 2, 1).reshape(HALF, 96))
        w = (radius[:, None] - d).astype(f32)
        nw = (neigh * w[:, :, None]).astype(f32)
        cov = np.matmul(nw.transpose(0, 2, 1), neigh).astype(f32)  # (HALF, 3, 3)
        cov /= (w.sum(1, dtype=f32)[:, None, None] + f32(EPS))
        cov6 = np.zeros((HALF, 8), f32)
        cov6[:, 0] = cov[:, 0, 0]
        cov6[:, 1] = cov[:, 0, 1]
        cov6[:, 2] = cov[:, 0, 2]
        cov6[:, 3] = cov[:, 1, 1]
        cov6[:, 4] = cov[:, 1, 2]
        cov6[:, 5] = cov[:, 2, 2]
        ppct = np.ascontiguousarray(
            (d.astype(f32) ** 2 + f32(CKEY)).astype(f32).T)       # (32, HALF)
        maps.append({"ngh": ngh, "cov6": cov6, "dd": np.ascontiguousarray(d),
                     "ppct": ppct, "gmat": G, "idn": idn})
        nbrs.append(nbr)
    return maps, nbrs


def host_assemble(p2_results, nbrs, template):
    """Decode k-slots, gather projections, barycentric weights, assemble."""
    template = np.asarray(template, f32)
    tmx = template[..., 0].reshape(NCELL).astype(f32)
    tmy = template[..., 1].reshape(NCELL).astype(f32)
    out = np.zeros((B, V, R, A, 3, 2), f32)
    rows = np.arange(HALF)[:, None, None]
    for core in range(8):
        b, h = core // 2, core % 2
        m3 = np.ascontiguousarray(p2_results[core]["m3o"])        # (HALF, 40, 3)
        k3 = (m3.view(np.int32) & 31).astype(np.int64)            # (HALF, 40, 3)
        pxy = p2_results[core]["pxy"]                             # (HALF, 2, 32)
        px = np.ascontiguousarray(pxy[:, 0, :])
        py = np.ascontiguousarray(pxy[:, 1, :])
        pxs = px[rows, k3]                                        # (HALF, 40, 3)
        pys = py[rows, k3]
        p0x, p1x, p2x = pxs[..., 0], pxs[..., 1], pxs[..., 2]
        p0y, p1y, p2y = pys[..., 0], pys[..., 1], pys[..., 2]
        v0x, v0y = p2x - p0x, p2y - p0y
        v1x, v1y = p1x - p0x, p1y - p0y
        v2x, v2y = tmx[None, :] - p0x, tmy[None, :] - p0y
        d00 = v0x * v0x + v0y * v0y
        d01 = v0x * v1x + v0y * v1y
        d02 = v0x * v2x + v0y * v2y
        d11 = v1x * v1x + v1y * v1y
        d12 = v1x * v2x + v1y * v2y
        den = d00 * d11 - d01 * d01 + f32(1e-6)
        w2 = (d11 * d02 - d01 * d12) / den
        w1 = (d00 * d12 - d01 * d02) / den
        w0 = f32(1.0) - w2 - w1
        weights = np.stack([w2, w1, w0], axis=-1)                 # (HALF, 40, 3)
        pidx = nbrs[core][rows[..., 0], k3.reshape(HALF, -1)].reshape(HALF, NCELL, 3)
        sl = slice(h * HALF, (h + 1) * HALF)
        out[b, sl, ..., 0] = pidx.reshape(HALF, R, A, 3).astype(f32)
        out[b, sl, ..., 1] = weights.reshape(HALF, R, A, 3).astype(f32)
    return out


_PROGS = {}


def _prog(name):
    if name not in _PROGS:
        _PROGS[name] = build_phase1() if name == "p1" else build_phase2()
    return _PROGS[name]


def run_phase1(vertices, trace=False):
    maps = host_prep_phase1(vertices)
    return run_bass_kernel_spmd(_prog("p1"), maps, list(range(8)), trace=trace)


def kernel(vertices, template, trace=False, _timing=None):
    vertices = np.asarray(vertices, f32)
    template = np.asarray(template, f32)
    r1 = run_bass_kernel_spmd(_prog("p1"), host_prep_phase1(vertices),
                              list(range(8)), trace=trace)
    maps2, nbrs = host_prep_phase2(vertices, template, r1.results)
    r2 = run_bass_kernel_spmd(_prog("p2"), maps2, list(range(8)), trace=trace)
    if _timing is not None:
        _timing["phase1"] = r1
        _timing["phase2"] = r2
        _timing["maps2"] = maps2
        _timing["nbrs"] = nbrs
    return host_assemble(r2.results, nbrs, template)


if __name__ == "__main__":
    # Phase-1 standalone check: exact top-33 coverage vs numpy brute force.
    cache = np.load("/root/problem/dev_cache/ref.npz")
    vertices = cache["vertices"]
    res = run_phase1(vertices)
    nbad = 0
    for core in range(8):
        b, h = core // 2, core % 2
        verts = np.ascontiguousarray(vertices[b], dtype=f32)
        nbr, d, rad = host_merge(res.results[core]["cand"], verts, h)
        # numpy exact reference
        sq = (verts * verts).sum(-1, dtype=f32).astype(f32)
        Q = verts[h * HALF:(h + 1) * HALF]
        d2full = (sq[None, :] + sq[h * HALF:(h + 1) * HALF, None]
                  - 2.0 * (Q @ verts.T)).astype(f32)
        order = np.lexsort((np.broadcast_to(np.arange(V), d2full.shape), d2full),
                           axis=1)[:, :33]
        miss = (np.sort(nbr, 1) != np.sort(order[:, :32], 1)).sum()
        print(f"core {core}: top32 mismatches={miss}")
        nbad += miss
    print("total nbr mismatches vs numpy exact:", nbad)
